# revision 17
# baseline (speedup 1.0000x reference)
"""Trainium2 Bass kernel: 3x3 VALID conv2d, stride 1.

Full input [32, 64, 112, 112] f32 + weights [128, 64, 3, 3] f32
-> output [32, 128, 110, 110] f32.

Data-parallel across 8 NeuronCores: 4 images per core.

Per-core formulation: conv as PE matmuls, out = lhsT.T @ rhs with
K (contraction, partitions) = 128 = 64 channels x 2 shifted copies,
M (out partitions) = 128 output channels,
N (moving free dim) = up to 4 input-width rows = 448 (<= 512, one PSUM
bank). The 2 rightmost columns of each 112-wide row are conv garbage;
the PSUM->SBUF copy compacts to the valid 110 columns.

FIVE matmuls per chunk cover all 9 taps (vs 6 for the naive row-pair
scheme) using two per-image SBUF planes:
  T tile: partitions 0-63 = image rows 0..110 (A), 64-127 = rows
          1..111 (B = A shifted one row).  Matmuls m=0..2 at column
          offset kx apply tap pairs (0,kx)+(1,kx).
  U tile: partitions 0-63 = rows 2..111 (C), 64-127 = rows 2..111
          shifted one column (C+1col).  m=3 applies (2,0)+(2,1) in one
          full-K matmul; m=4 applies (2,2) on the top half only
          (bottom-half weights zero) -- the single unavoidable
          half-waste (9 taps is odd).
U is built on-device by per-band COMPUTE-ENGINE copies (not DMA): the
lo half is T partitions 0-63 shifted +224 elements (A extended to all
112 rows) and the hi half is T partitions 64-127 shifted +113, i.e.
both are same-partition contiguous copies that vector/scalar/gpsimd
tensor-copy at full engine rate, off the DMA fabric.  HBM input
traffic stays at the single-copy ~12.9 MB/core.
(Alternatives measured and rejected: streaming both planes from HBM
saturates the ~358 GB/s HBM interface and starves the PE; building U
via SBUF->SBUF DMA serializes on one HWDGE ring at ~125 GB/s and
starves the m=3/4 matmuls.)

Moving-N per tap is trimmed (n, n-1, n-2, n-1, n-2) so no rhs read
spills past input row y0+3: only garbage output columns lose taps.

Inputs are cast to fp16 on the host (fp32 PE is 4x slower; fp32 PSUM
accumulation keeps rel err ~3e-4).  Output is stored fp16 and cast
back to fp32 on the host, halving output HBM traffic.

A short burst of dummy matmuls on a memset tile runs during the DMA
startup window to flip the PE HAM clock gate to 2.4 GHz before real
work arrives.

Schedule: chunks are processed in groups of 8 across the 8 PSUM banks,
weight-plane-major (m outer), so consecutive matmuls hit different
banks (drain overlaps fill) and reuse the same stationary weights.
Engine split: gpsimd issues x loads, scalar builds U, vector does the
PSUM compaction copies (scalar helps on the last groups), sync does
the per-2-chunk output DMAs.
"""

import numpy as np

B_FULL = 32
N_CORES = 8
B_CORE = B_FULL // N_CORES  # 4 images per core
C_IN = 64
C_OUT = 128
H = W = 112
OH = OW = 110
TLEN = 112 * W  # 12544: T plane rows 0..111 (A) / 1..111 + zero pad (B)
ULEN = 110 * W  # 12320: U plane rows 2..111

_NC = None


def _img_chunks():
    # per image: 27 chunks of 4 output rows + 1 of 2 rows = 110
    rows_list = [4] * 27 + [2]
    out = []
    y0 = 0
    for r in rows_list:
        out.append((y0, r))
        y0 += r
    assert y0 == OH
    return out


def _build():
    from contextlib import ExitStack

    import concourse.tile as tile
    from concourse import bacc, mybir

    nc = bacc.Bacc("TRN2", target_bir_lowering=False, debug=False)
    x = nc.dram_tensor(
        "x", [B_CORE, 128, TLEN], mybir.dt.float16, kind="ExternalInput"
    )
    w = nc.dram_tensor("w", [128, 5, 128], mybir.dt.float16, kind="ExternalInput")
    y = nc.dram_tensor(
        "y", [B_CORE, C_OUT, OH * OW], mybir.dt.float16, kind="ExternalOutput"
    )

    chunks = [(b, y0, r) for b in range(B_CORE) for (y0, r) in _img_chunks()]
    assert len(chunks) % 8 == 0
    n_groups = len(chunks) // 8

    with tile.TileContext(nc) as tc, ExitStack() as ctx:
        tpool = ctx.enter_context(tc.tile_pool(name="tp", bufs=B_CORE))
        upool = ctx.enter_context(tc.tile_pool(name="up", bufs=B_CORE))
        wpool = ctx.enter_context(tc.tile_pool(name="wp", bufs=1))
        spool = ctx.enter_context(tc.tile_pool(name="sp", bufs=1))
        opool = ctx.enter_context(tc.tile_pool(name="op", bufs=6))
        ppool = ctx.enter_context(tc.tile_pool(name="pp", bufs=8, space="PSUM"))

        wt = wpool.tile([128, 5 * 128], mybir.dt.float16)
        nc.sync.dma_start(wt[:], w.ap().rearrange("p a b -> p (a b)"))

        # PE warmup: HAM clock gate flips to 2.4 GHz after ~3.4us of
        # sustained activity; burn that in while the first x bands load.
        wu = spool.tile([128, 128], mybir.dt.float16)
        nc.gpsimd.memset(wu[:], 0)
        wu_p = ppool.tile([128, 448], mybir.dt.float32, name="wu_p", tag="pt")
        for _ in range(14):
            nc.tensor.matmul(
                wu_p[0:64, 0:128], wu[:, 0:64], wu[:],
                start=True, stop=True, skip_group_check=True,
            )

        xa = x.ap()
        ya = y.ap()

        # Banded loads so the first chunks start early.  Image 0's first
        # bands ride the sync queue (earliest to start); the bulk of T
        # streams on gpsimd.  U band k reads only T band k (U edges =
        # T edges - 2).  U-hi copies run on gpsimd, interleaved with the
        # T dma issues so a sem wait never delays the T stream by much;
        # U-lo copies are emitted in the group loop just before first
        # use (vector/scalar alternating) so they never head-block the
        # output casts behind an unfired T semaphore.
        t_tiles = [
            tpool.tile([128, TLEN], mybir.dt.float16, name=f"t{b}", tag="t")
            for b in range(B_CORE)
        ]
        u_tiles = [
            upool.tile([128, ULEN], mybir.dt.float16, name=f"u{b}", tag="u")
            for b in range(B_CORE)
        ]
        TBs = [[0, 6, 16, 34, 61, 89, 112]] + [[0, 16, 34, 61, 89, 112]] * 3
        UBs = [[0, 4, 14, 32, 59, 87, 110]] + [[0, 14, 32, 59, 87, 110]] * 3

        def t_issue(b, k):
            lo, hi = TBs[b][k], TBs[b][k + 1]
            eng = nc.sync if (b == 0 and k < 2) else nc.gpsimd
            eng.dma_start(
                t_tiles[b][:, lo * W : hi * W], xa[b][:, lo * W : hi * W]
            )

        def u_hi(b, k):
            lo, hi = UBs[b][k], UBs[b][k + 1]
            nc.gpsimd.tensor_copy(
                u_tiles[b][64:128, lo * W : hi * W],
                t_tiles[b][64:128, lo * W + 113 : hi * W + 113],
            )

        def u_lo(b, k, i):
            lo, hi = UBs[b][k], UBs[b][k + 1]
            eng = nc.vector.tensor_copy if i % 2 == 0 else nc.scalar.copy
            eng(
                u_tiles[b][0:64, lo * W : hi * W],
                t_tiles[b][0:64, (lo + 2) * W : (hi + 2) * W],
            )

        for k in range(6):
            t_issue(0, k)
        u_hi(0, 0), u_hi(0, 1)
        for k in range(5):
            t_issue(1, k)
        for k in range(2, 6):
            u_hi(0, k)
        for k in range(5):
            t_issue(2, k)
        u_hi(1, 0), u_hi(1, 1)
        for k in range(5):
            t_issue(3, k)
        for k in range(2, 5):
            u_hi(1, k)
        for b in (2, 3):
            for k in range(5):
                u_hi(b, k)

        # group index -> U-lo bands first needed one group later
        ULO_SCHED = {
            -1: [(0, 0), (0, 1), (0, 2)],
            0: [(0, 3), (0, 4)],
            1: [(0, 5)],
            2: [(1, 0), (1, 1)],
            3: [(1, 2)],
            5: [(1, 3), (1, 4)],
            6: [(2, 0), (2, 1)],
            7: [(2, 2)],
            8: [(2, 3)],
            9: [(2, 4), (3, 0), (3, 1)],
            10: [(3, 2)],
            12: [(3, 3), (3, 4)],
        }
        ulo_i = 0
        for b, k in ULO_SCHED[-1]:
            u_lo(b, k, ulo_i)
            ulo_i += 1

        for g in range(n_groups):
            for b, k in ULO_SCHED.get(g, []):
                u_lo(b, k, ulo_i)
                ulo_i += 1
            gchunks = chunks[g * 8 : (g + 1) * 8]
            pts = [
                ppool.tile([128, 448], mybir.dt.float32, name="pt", tag="pt")
                for _ in range(8)
            ]
            for m in range(5):
                for j, (b, y0, rows) in enumerate(gchunks):
                    n = rows * W
                    if m < 3:
                        nmv = n - m
                        rhs = t_tiles[b][:, y0 * W + m : y0 * W + m + nmv]
                    elif m == 3:
                        nmv = n - 1
                        rhs = u_tiles[b][:, y0 * W : y0 * W + nmv]
                    else:
                        nmv = n - 2
                        rhs = u_tiles[b][:, y0 * W + 1 : y0 * W + 1 + nmv]
                    nc.tensor.matmul(
                        pts[j][:, 0:nmv],
                        wt[:, m * 128 : (m + 1) * 128],
                        rhs,
                        start=(m == 0),
                        stop=(m == 4),
                        skip_group_check=True,
                    )
            # compact + store per 2 chunks: copies start draining PSUM as
            # soon as each pair of banks stops; ~0.22MB DMAs keep HWDGE
            # efficient without delaying the tail
            for h in range(4):
                pair = gchunks[2 * h : 2 * h + 2]
                total_rows = sum(r for _, _, r in pair)
                ot = opool.tile([128, 8 * OW], mybir.dt.float16, tag="ot")
                off = 0
                for jj, (b, y0, rows) in enumerate(pair):
                    j = 2 * h + jj
                    psrc = pts[j][:].rearrange("p (r c) -> p r c", c=W)[
                        :, 0:rows, 0:OW
                    ]
                    odst = ot[:, off : off + rows * OW].rearrange(
                        "p (r c) -> p r c", c=OW
                    )
                    if jj == 0:
                        nc.vector.tensor_copy(odst, psrc)
                    else:
                        nc.scalar.copy(odst, psrc)
                    off += rows * OW
                b0, y00, _ = pair[0]
                assert all(b == b0 for b, _, _ in pair)
                nc.sync.dma_start(
                    ya[b0][:, y00 * OW : y00 * OW + total_rows * OW],
                    ot[:, 0 : total_rows * OW],
                )

    nc.compile()
    return nc


def _get_nc():
    global _NC
    if _NC is None:
        _NC = _build()
    return _NC


def _prep_weights(weights: np.ndarray) -> np.ndarray:
    # w5[c,     m, co] = w[co, c, 0, m] (m<3) ; w5[c,    3, co] = w[co, c, 2, 0]
    # w5[64+c,  m, co] = w[co, c, 1, m] (m<3) ; w5[64+c, 3, co] = w[co, c, 2, 1]
    # w5[c, 4, co] = 0                        ; w5[64+c, 4, co] = w[co, c, 2, 2]
    w = np.asarray(weights, dtype=np.float32)
    wt = w.transpose(1, 2, 3, 0)  # [ci, ky, kx, co]
    w5 = np.zeros((128, 5, 128), np.float32)
    w5[0:64, 0:3, :] = wt[:, 0, :, :]
    w5[64:128, 0:3, :] = wt[:, 1, :, :]
    w5[0:64, 3, :] = wt[:, 2, 0, :]
    w5[64:128, 3, :] = wt[:, 2, 1, :]
    w5[64:128, 4, :] = wt[:, 2, 2, :]
    return w5.astype(np.float16)


def kernel(input_image: np.ndarray, weights: np.ndarray, _trace: bool = False):
    from concourse.bass_utils import run_bass_kernel_spmd

    nc = _get_nc()
    x16 = np.asarray(input_image).astype(np.float16)
    r = x16.reshape(B_FULL, C_IN, H * W)
    xd = np.zeros((B_FULL, 128, TLEN), np.float16)
    xd[:, 0:64] = r  # A: rows 0..111
    xd[:, 64:128, : TLEN - W] = r[:, :, W:]  # B: rows 1..111, zero pad
    w5 = _prep_weights(weights)
    in_maps = [
        {"x": xd[B_CORE * i : B_CORE * (i + 1)], "w": w5} for i in range(N_CORES)
    ]
    res = run_bass_kernel_spmd(
        nc, in_maps, core_ids=list(range(N_CORES)), trace=_trace
    )
    out = np.concatenate([res.results[i]["y"] for i in range(N_CORES)], axis=0)
    out = out.reshape(B_FULL, C_OUT, OH, OW).astype(np.float32)
    if _trace:
        return out, res
    return out


# revision 19
# speedup vs baseline: 1.7101x; 1.7101x over previous
"""Trainium2 Bass kernel: 3x3 VALID conv2d, stride 1.

Full input [32, 64, 112, 112] f32 + weights [128, 64, 3, 3] f32
-> output [32, 128, 110, 110] f32.

Data-parallel across 8 NeuronCores: 4 images per core.

Per-core formulation: conv as PE matmuls, out = lhsT.T @ rhs with
K (contraction, partitions) = 128 = 64 channels x 2 shifted copies,
M (out partitions) = 128 output channels,
N (moving free dim) = up to 4 input-width rows = 448 (<= 512, one PSUM
bank). The 2 rightmost columns of each 112-wide row are conv garbage;
the PSUM->SBUF copy compacts to the valid 110 columns.

Tap coverage per chunk, two schemes:
  T tile (all chunks): partitions 0-63 = image rows 0..111 (A),
          64-127 = rows 1..111 (B).  Matmuls m=0..2 at column offset
          kx apply tap pairs (0,kx)+(1,kx).
  5-MM scheme (chunks with y0 < 84): U tile: partitions 0-63 = rows
          2..88 (C), 64-127 = same shifted one column.  m=3 applies
          (2,0)+(2,1) in one full-K matmul; m=4 applies (2,2) on the
          hi half only -- 5 matmuls for 9 taps.
  6-MM scheme (y0 >= 84): ky=2 taps via T at row offset +1 with
          zero weights on the A half (planes 5-7) -- 6 matmuls, no U.

U is built on-device by two same-partition contiguous SBUF->SBUF DMAs
per band (lo: A shifted +224 elements; hi: B shifted +113), so HBM
input traffic stays at the single-copy ~12.9 MB/core.  Measured mover
rates force the hybrid: one HWDGE ring moves ~125 GB/s serialized, and
full-5MM needs ~140 GB/s of U-build; 75% coverage fits two rings.
U-lo copies ride the scalar ring, U-hi the sync ring, and the output
DMAs alternate between both rings to balance load; all U copies are
emitted in the group loop just before first use so an unfired T-band
semaphore never head-blocks other work on the queue.
(Also measured and rejected: streaming U from HBM saturates the
~358 GB/s HBM interface; compute-engine tensor_copy builds run at a
crawl, 14-25 G elem/s.)

Moving-N per tap is trimmed (n, n-1, n-2, ...) so no rhs read spills
past input row y0+3 (y0+4 for the 6-MM tail rows): only garbage output
columns lose taps.

Inputs are cast to fp16 on the host (fp32 PE is 4x slower; fp32 PSUM
accumulation keeps rel err ~4e-4).  Output is stored fp16 and cast
back to fp32 on the host, halving output HBM traffic.

A short burst of dummy matmuls on a memset tile runs during the DMA
startup window so the PE HAM clock gate flips to 2.4 GHz by the time
real work arrives.

Schedule: chunks are processed in groups of 8 across the 8 PSUM banks,
weight-plane-major (m outer), so consecutive matmuls hit different
banks (drain overlaps fill).  PSUM compaction casts alternate
vector/scalar per chunk.
"""

import numpy as np

B_FULL = 32
N_CORES = 8
B_CORE = B_FULL // N_CORES  # 4 images per core
C_IN = 64
C_OUT = 128
H = W = 112
OH = OW = 110
TLEN = 112 * W  # T plane: rows 0..111 (A) / 1..111 + zero pad (B)
Y5 = 84  # chunks with y0 < Y5 use the 5-matmul scheme
UROWS = Y5 + 3  # U plane rows 2..88
ULEN = UROWS * W

_NC = None


def _img_chunks():
    # per image: 27 chunks of 4 output rows + 1 of 2 rows = 110
    rows_list = [4] * 27 + [2]
    out = []
    y0 = 0
    for r in rows_list:
        out.append((y0, r))
        y0 += r
    assert y0 == OH
    return out


def _build():
    from contextlib import ExitStack

    import concourse.tile as tile
    from concourse import bacc, mybir

    nc = bacc.Bacc("TRN2", target_bir_lowering=False, debug=False)
    x = nc.dram_tensor(
        "x", [B_CORE, 128, TLEN], mybir.dt.float16, kind="ExternalInput"
    )
    w = nc.dram_tensor("w", [128, 8, 128], mybir.dt.float16, kind="ExternalInput")
    y = nc.dram_tensor(
        "y", [B_CORE, C_OUT, OH * OW], mybir.dt.float16, kind="ExternalOutput"
    )

    chunks = [(b, y0, r) for b in range(B_CORE) for (y0, r) in _img_chunks()]
    assert len(chunks) % 8 == 0
    n_groups = len(chunks) // 8

    with tile.TileContext(nc) as tc, ExitStack() as ctx:
        tpool = ctx.enter_context(tc.tile_pool(name="tp", bufs=B_CORE))
        upool = ctx.enter_context(tc.tile_pool(name="up", bufs=B_CORE))
        wpool = ctx.enter_context(tc.tile_pool(name="wp", bufs=1))
        spool = ctx.enter_context(tc.tile_pool(name="sp", bufs=1))
        opool = ctx.enter_context(tc.tile_pool(name="op", bufs=6))
        ppool = ctx.enter_context(tc.tile_pool(name="pp", bufs=8, space="PSUM"))

        wt = wpool.tile([128, 8 * 128], mybir.dt.float16)
        nc.sync.dma_start(wt[:], w.ap().rearrange("p a b -> p (a b)"))

        # PE warmup: HAM clock gate flips to 2.4 GHz after ~3.4us of
        # sustained activity; burn that in while the first x bands load.
        wu = spool.tile([128, 128], mybir.dt.float16)
        nc.gpsimd.memset(wu[:], 0)
        wu_p = ppool.tile([128, 448], mybir.dt.float32, name="wu_p", tag="pt")
        for _ in range(14):
            nc.tensor.matmul(
                wu_p[0:64, 0:128], wu[:, 0:64], wu[:],
                start=True, stop=True, skip_group_check=True,
            )

        xa = x.ap()
        ya = y.ap()

        # Banded loads so the first chunks start early.  Image 0's first
        # two T bands ride the sync queue (earliest to start); the bulk
        # of T streams on gpsimd/SWDGE.  U band k reads only T band k
        # (U edges = T edges - 2).
        t_tiles = [
            tpool.tile([128, TLEN], mybir.dt.float16, name=f"t{b}", tag="t")
            for b in range(B_CORE)
        ]
        u_tiles = [
            upool.tile([128, ULEN], mybir.dt.float16, name=f"u{b}", tag="u")
            for b in range(B_CORE)
        ]
        TBs = [[0, 6, 16, 34, 61, 89, 112]] + [[0, 16, 34, 61, 89, 112]] * 3
        UBs = [[0, 4, 14, 32, 59, UROWS]] + [[0, 14, 32, 59, UROWS]] * 3

        def t_issue(b, k):
            lo, hi = TBs[b][k], TBs[b][k + 1]
            eng = nc.sync if (b == 0 and k < 2) else nc.gpsimd
            eng.dma_start(
                t_tiles[b][:, lo * W : hi * W], xa[b][:, lo * W : hi * W]
            )

        def u_build(b, k):
            lo, hi = UBs[b][k], UBs[b][k + 1]
            nc.scalar.dma_start(
                u_tiles[b][0:64, lo * W : hi * W],
                t_tiles[b][0:64, (lo + 2) * W : (hi + 2) * W],
            )
            nc.sync.dma_start(
                u_tiles[b][64:128, lo * W : hi * W],
                t_tiles[b][64:128, lo * W + 113 : hi * W + 113],
            )

        for b in range(B_CORE):
            for k in range(len(TBs[b]) - 1):
                t_issue(b, k)

        # group index -> U bands first needed one group later
        ULO_SCHED = {
            -1: [(0, 0), (0, 1), (0, 2)],
            0: [(0, 3), (0, 4)],
            2: [(1, 0), (1, 1)],
            3: [(1, 2)],
            4: [(1, 3)],
            6: [(2, 0), (2, 1)],
            7: [(2, 2), (2, 3)],
            9: [(3, 0), (3, 1)],
            10: [(3, 2)],
            11: [(3, 3)],
        }
        for b, k in ULO_SCHED[-1]:
            u_build(b, k)

        def chunk_taps(b, y0, rows):
            n = rows * W
            t, u = t_tiles[b], u_tiles[b]
            if y0 < Y5:  # 5-matmul scheme
                return [
                    (0, t, y0 * W, n),
                    (1, t, y0 * W + 1, n - 1),
                    (2, t, y0 * W + 2, n - 2),
                    (3, u, y0 * W, n - 1),
                    (4, u, y0 * W + 1, n - 2),
                ]
            return [  # 6-matmul scheme, ky=2 via T at +1 row
                (0, t, y0 * W, n),
                (1, t, y0 * W + 1, n - 1),
                (2, t, y0 * W + 2, n - 2),
                (5, t, (y0 + 1) * W, n),
                (6, t, (y0 + 1) * W + 1, n - 1),
                (7, t, (y0 + 1) * W + 2, n - 2),
            ]

        for g in range(n_groups):
            for b, k in ULO_SCHED.get(g, []):
                u_build(b, k)
            gchunks = chunks[g * 8 : (g + 1) * 8]
            pts = [
                ppool.tile([128, 448], mybir.dt.float32, name="pt", tag="pt")
                for _ in range(8)
            ]
            taps = [chunk_taps(*c) for c in gchunks]
            for m in range(8):
                for j in range(8):
                    for mi, (mm, src, off, nmv) in enumerate(taps[j]):
                        if mm != m:
                            continue
                        nc.tensor.matmul(
                            pts[j][:, 0:nmv],
                            wt[:, m * 128 : (m + 1) * 128],
                            src[:, off : off + nmv],
                            start=(mi == 0),
                            stop=(mi == len(taps[j]) - 1),
                            skip_group_check=True,
                        )
            # compact + store per 2 chunks: copies start draining PSUM as
            # soon as each pair of banks stops; out DMAs alternate between
            # the sync and scalar rings to balance ring load
            for h in range(4):
                pair = gchunks[2 * h : 2 * h + 2]
                total_rows = sum(r for _, _, r in pair)
                ot = opool.tile([128, 8 * OW], mybir.dt.float16, tag="ot")
                off = 0
                for jj, (b, y0, rows) in enumerate(pair):
                    j = 2 * h + jj
                    psrc = pts[j][:].rearrange("p (r c) -> p r c", c=W)[
                        :, 0:rows, 0:OW
                    ]
                    odst = ot[:, off : off + rows * OW].rearrange(
                        "p (r c) -> p r c", c=OW
                    )
                    if jj == 0:
                        nc.vector.tensor_copy(odst, psrc)
                    else:
                        nc.scalar.copy(odst, psrc)
                    off += rows * OW
                b0, y00, _ = pair[0]
                assert all(b == b0 for b, _, _ in pair)
                oeng = nc.sync if h % 2 == 0 else nc.scalar
                oeng.dma_start(
                    ya[b0][:, y00 * OW : y00 * OW + total_rows * OW],
                    ot[:, 0 : total_rows * OW],
                )

    nc.compile()
    return nc


def _get_nc():
    global _NC
    if _NC is None:
        _NC = _build()
    return _NC


def _prep_weights(weights: np.ndarray) -> np.ndarray:
    # planes 0-2: rows 0-63 = taps (0,m), rows 64-127 = taps (1,m)
    # plane 3: (2,0) | (2,1); plane 4: zero | (2,2)     [5-MM scheme]
    # planes 5-7: zero | (2,kx)                          [6-MM scheme]
    w = np.asarray(weights, dtype=np.float32)
    wt = w.transpose(1, 2, 3, 0)  # [ci, ky, kx, co]
    w8 = np.zeros((128, 8, 128), np.float32)
    w8[0:64, 0:3, :] = wt[:, 0, :, :]
    w8[64:128, 0:3, :] = wt[:, 1, :, :]
    w8[0:64, 3, :] = wt[:, 2, 0, :]
    w8[64:128, 3, :] = wt[:, 2, 1, :]
    w8[64:128, 4, :] = wt[:, 2, 2, :]
    w8[64:128, 5:8, :] = wt[:, 2, :, :]
    return w8.astype(np.float16)


def kernel(input_image: np.ndarray, weights: np.ndarray, _trace: bool = False):
    from concourse.bass_utils import run_bass_kernel_spmd

    nc = _get_nc()
    x16 = np.asarray(input_image).astype(np.float16)
    r = x16.reshape(B_FULL, C_IN, H * W)
    xd = np.zeros((B_FULL, 128, TLEN), np.float16)
    xd[:, 0:64] = r  # A: rows 0..111
    xd[:, 64:128, : TLEN - W] = r[:, :, W:]  # B: rows 1..111, zero pad
    w8 = _prep_weights(weights)
    in_maps = [
        {"x": xd[B_CORE * i : B_CORE * (i + 1)], "w": w8} for i in range(N_CORES)
    ]
    res = run_bass_kernel_spmd(
        nc, in_maps, core_ids=list(range(N_CORES)), trace=_trace
    )
    out = np.concatenate([res.results[i]["y"] for i in range(N_CORES)], axis=0)
    out = out.reshape(B_FULL, C_OUT, OH, OW).astype(np.float32)
    if _trace:
        return out, res
    return out


# revision 23
# speedup vs baseline: 1.7912x; 1.0474x over previous
"""Trainium2 Bass kernel: 3x3 VALID conv2d, stride 1.

Full input [32, 64, 112, 112] f32 + weights [128, 64, 3, 3] f32
-> output [32, 128, 110, 110] f32.

Data-parallel across 8 NeuronCores: 4 images per core.

Per-core formulation: conv as PE matmuls, out = lhsT.T @ rhs with
K (contraction, partitions) = 128 = 64 channels x 2 shifted copies,
M (out partitions) = 128 output channels,
N (moving free dim) = up to 4 input-width rows = 448 (<= 512, one PSUM
bank). The 2 rightmost columns of each 112-wide row are conv garbage;
the PSUM->SBUF copy compacts to the valid 110 columns.

Tap coverage per chunk, two schemes:
  T tile (all chunks): partitions 0-63 = image rows 0..111 (A),
          64-127 = rows 1..111 (B).  Matmuls m=0..2 at column offset
          kx apply tap pairs (0,kx)+(1,kx).
  5-MM scheme (chunks with y0 < 84): U tile: partitions 0-63 = rows
          2..88 (C), 64-127 = same shifted one column.  m=3 applies
          (2,0)+(2,1) in one full-K matmul; m=4 applies (2,2) on the
          hi half only -- 5 matmuls for 9 taps.
  6-MM scheme (y0 >= 84): ky=2 taps via T at row offset +1 with
          zero weights on the A half (planes 5-7) -- 6 matmuls, no U.

U is built on-device by two same-partition contiguous SBUF->SBUF DMAs
per band (lo: A shifted +224 elements; hi: B shifted +113), so HBM
input traffic stays at the single-copy ~12.9 MB/core.  Measured mover
rates force the hybrid: one HWDGE ring moves ~125 GB/s serialized, and
full-5MM needs ~140 GB/s of U-build; 75% coverage fits two rings.
U-lo copies ride the scalar ring, U-hi the sync ring, and the output
DMAs alternate between both rings to balance load; all U copies are
emitted in the group loop just before first use so an unfired T-band
semaphore never head-blocks other work on the queue.
(Also measured and rejected: streaming U from HBM saturates the
~358 GB/s HBM interface; compute-engine tensor_copy builds run at a
crawl, 14-25 G elem/s.)

Moving-N per tap is trimmed (n, n-1, n-2, ...) so no rhs read spills
past input row y0+3 (y0+4 for the 6-MM tail rows): only garbage output
columns lose taps.

Inputs are cast to fp16 on the host (fp32 PE is 4x slower; fp32 PSUM
accumulation keeps rel err ~4e-4).  Output is stored fp16 and cast
back to fp32 on the host, halving output HBM traffic.

A short burst of dummy matmuls on a memset tile runs during the DMA
startup window so the PE HAM clock gate flips to 2.4 GHz by the time
real work arrives.

Schedule: chunks are processed in groups of 8 across the 8 PSUM banks,
weight-plane-major (m outer), so consecutive matmuls hit different
banks (drain overlaps fill).  PSUM compaction casts alternate
vector/scalar per chunk.
"""

import numpy as np

B_FULL = 32
N_CORES = 8
B_CORE = B_FULL // N_CORES  # 4 images per core
C_IN = 64
C_OUT = 128
H = W = 112
OH = OW = 110
TLEN = 112 * W  # T plane: rows 0..111 (A) / 1..111 + zero pad (B)
Y5 = 84  # chunks with y0 < Y5 use the 5-matmul scheme
UROWS = Y5 + 3  # U plane rows 2..88
ULEN = UROWS * W

_NC = None


def _img_chunks():
    # per image: 27 chunks of 4 output rows + 1 of 2 rows = 110
    rows_list = [4] * 27 + [2]
    out = []
    y0 = 0
    for r in rows_list:
        out.append((y0, r))
        y0 += r
    assert y0 == OH
    return out


def _build():
    from contextlib import ExitStack

    import concourse.tile as tile
    from concourse import bacc, mybir

    nc = bacc.Bacc("TRN2", target_bir_lowering=False, debug=False)
    x = nc.dram_tensor(
        "x", [B_CORE, 128, TLEN], mybir.dt.float16, kind="ExternalInput"
    )
    w = nc.dram_tensor("w", [128, 8, 128], mybir.dt.float16, kind="ExternalInput")
    y = nc.dram_tensor(
        "y", [B_CORE, C_OUT, OH * OW], mybir.dt.float16, kind="ExternalOutput"
    )

    chunks = [(b, y0, r) for b in range(B_CORE) for (y0, r) in _img_chunks()]
    assert len(chunks) % 8 == 0
    n_groups = len(chunks) // 8

    with tile.TileContext(nc) as tc, ExitStack() as ctx:
        tpool = ctx.enter_context(tc.tile_pool(name="tp", bufs=B_CORE))
        upool = ctx.enter_context(tc.tile_pool(name="up", bufs=B_CORE))
        wpool = ctx.enter_context(tc.tile_pool(name="wp", bufs=1))
        spool = ctx.enter_context(tc.tile_pool(name="sp", bufs=1))
        opool = ctx.enter_context(tc.tile_pool(name="op", bufs=12))
        ppool = ctx.enter_context(tc.tile_pool(name="pp", bufs=8, space="PSUM"))

        wt = wpool.tile([128, 8 * 128], mybir.dt.float16)
        nc.sync.dma_start(wt[:], w.ap().rearrange("p a b -> p (a b)"))

        # PE warmup: HAM clock gate flips to 2.4 GHz after ~3.4us of
        # sustained activity; burn that in while the first x bands load.
        wu = spool.tile([128, 128], mybir.dt.float16)
        nc.gpsimd.memset(wu[:], 0)
        wu_p = ppool.tile([128, 448], mybir.dt.float32, name="wu_p", tag="pt")
        for _ in range(14):
            nc.tensor.matmul(
                wu_p[0:64, 0:128], wu[:, 0:64], wu[:],
                start=True, stop=True, skip_group_check=True,
            )

        xa = x.ap()
        ya = y.ap()

        # Banded loads so the first chunks start early.  Image 0's first
        # two T bands ride the sync queue (earliest to start); the bulk
        # of T streams on gpsimd/SWDGE.  U band k reads only T band k
        # (U edges = T edges - 2).
        t_tiles = [
            tpool.tile([128, TLEN], mybir.dt.float16, name=f"t{b}", tag="t")
            for b in range(B_CORE)
        ]
        u_tiles = [
            upool.tile([128, ULEN], mybir.dt.float16, name=f"u{b}", tag="u")
            for b in range(B_CORE)
        ]
        TBs = [[0, 6, 16, 26, 34, 61, 89, 112]] + [[0, 16, 34, 61, 89, 112]] * 3
        UBs = [[0, 4, 14, 24, 32, 59, UROWS]] + [[0, 14, 32, 59, UROWS]] * 3

        def t_issue(b, k):
            lo, hi = TBs[b][k], TBs[b][k + 1]
            eng = nc.sync if (b == 0 and k < 3) else nc.gpsimd
            eng.dma_start(
                t_tiles[b][:, lo * W : hi * W], xa[b][:, lo * W : hi * W]
            )

        def u_build(b, k):
            lo, hi = UBs[b][k], UBs[b][k + 1]
            nc.scalar.dma_start(
                u_tiles[b][0:64, lo * W : hi * W],
                t_tiles[b][0:64, (lo + 2) * W : (hi + 2) * W],
            )
            nc.sync.dma_start(
                u_tiles[b][64:128, lo * W : hi * W],
                t_tiles[b][64:128, lo * W + 113 : hi * W + 113],
            )

        for b in range(B_CORE):
            for k in range(len(TBs[b]) - 1):
                t_issue(b, k)

        # group index -> U bands first needed one group later
        ULO_SCHED = {
            -1: [(0, 0), (0, 1), (0, 2), (0, 3)],
            0: [(0, 4), (0, 5)],
            2: [(1, 0), (1, 1)],
            3: [(1, 2)],
            4: [(1, 3)],
            6: [(2, 0), (2, 1)],
            7: [(2, 2), (2, 3)],
            9: [(3, 0), (3, 1)],
            10: [(3, 2)],
            11: [(3, 3)],
        }
        for b, k in ULO_SCHED[-1]:
            u_build(b, k)

        def chunk_taps(b, y0, rows):
            n = rows * W
            t, u = t_tiles[b], u_tiles[b]
            if y0 < Y5:  # 5-matmul scheme
                return [
                    (0, t, y0 * W, n),
                    (1, t, y0 * W + 1, n - 1),
                    (2, t, y0 * W + 2, n - 2),
                    (3, u, y0 * W, n - 1),
                    (4, u, y0 * W + 1, n - 2),
                ]
            return [  # 6-matmul scheme, ky=2 via T at +1 row
                (0, t, y0 * W, n),
                (1, t, y0 * W + 1, n - 1),
                (2, t, y0 * W + 2, n - 2),
                (5, t, (y0 + 1) * W, n),
                (6, t, (y0 + 1) * W + 1, n - 1),
                (7, t, (y0 + 1) * W + 2, n - 2),
            ]

        for g in range(n_groups):
            for b, k in ULO_SCHED.get(g, []):
                u_build(b, k)
            gchunks = chunks[g * 8 : (g + 1) * 8]
            pts = [
                ppool.tile([128, 448], mybir.dt.float32, name="pt", tag="pt")
                for _ in range(8)
            ]
            taps = [chunk_taps(*c) for c in gchunks]
            for m in range(8):
                for j in range(8):
                    for mi, (mm, src, off, nmv) in enumerate(taps[j]):
                        if mm != m:
                            continue
                        nc.tensor.matmul(
                            pts[j][:, 0:nmv],
                            wt[:, m * 128 : (m + 1) * 128],
                            src[:, off : off + nmv],
                            start=(mi == 0),
                            stop=(mi == len(taps[j]) - 1),
                            skip_group_check=True,
                        )
            # compact + store per 2 chunks: copies start draining PSUM as
            # soon as each pair of banks stops; out DMAs alternate between
            # the sync and scalar rings to balance ring load
            for h in range(4):
                pair = gchunks[2 * h : 2 * h + 2]
                total_rows = sum(r for _, _, r in pair)
                ot = opool.tile([128, 8 * OW], mybir.dt.float16, tag="ot")
                off = 0
                for jj, (b, y0, rows) in enumerate(pair):
                    j = 2 * h + jj
                    psrc = pts[j][:].rearrange("p (r c) -> p r c", c=W)[
                        :, 0:rows, 0:OW
                    ]
                    odst = ot[:, off : off + rows * OW].rearrange(
                        "p (r c) -> p r c", c=OW
                    )
                    # all casts on vector: it issues no DMAs, so PSUM
                    # drain is never head-blocked by an unfired T-band
                    # semaphore on a DMA-issuing queue
                    nc.vector.tensor_copy(odst, psrc)
                    off += rows * OW
                b0, y00, _ = pair[0]
                assert all(b == b0 for b, _, _ in pair)
                oeng = nc.sync if h % 2 == 0 else nc.scalar
                oeng.dma_start(
                    ya[b0][:, y00 * OW : y00 * OW + total_rows * OW],
                    ot[:, 0 : total_rows * OW],
                )

    nc.compile()
    return nc


def _get_nc():
    global _NC
    if _NC is None:
        _NC = _build()
    return _NC


def _prep_weights(weights: np.ndarray) -> np.ndarray:
    # planes 0-2: rows 0-63 = taps (0,m), rows 64-127 = taps (1,m)
    # plane 3: (2,0) | (2,1); plane 4: zero | (2,2)     [5-MM scheme]
    # planes 5-7: zero | (2,kx)                          [6-MM scheme]
    w = np.asarray(weights, dtype=np.float32)
    wt = w.transpose(1, 2, 3, 0)  # [ci, ky, kx, co]
    w8 = np.zeros((128, 8, 128), np.float32)
    w8[0:64, 0:3, :] = wt[:, 0, :, :]
    w8[64:128, 0:3, :] = wt[:, 1, :, :]
    w8[0:64, 3, :] = wt[:, 2, 0, :]
    w8[64:128, 3, :] = wt[:, 2, 1, :]
    w8[64:128, 4, :] = wt[:, 2, 2, :]
    w8[64:128, 5:8, :] = wt[:, 2, :, :]
    return w8.astype(np.float16)


def kernel(input_image: np.ndarray, weights: np.ndarray, _trace: bool = False):
    from concourse.bass_utils import run_bass_kernel_spmd

    nc = _get_nc()
    x16 = np.asarray(input_image).astype(np.float16)
    r = x16.reshape(B_FULL, C_IN, H * W)
    xd = np.zeros((B_FULL, 128, TLEN), np.float16)
    xd[:, 0:64] = r  # A: rows 0..111
    xd[:, 64:128, : TLEN - W] = r[:, :, W:]  # B: rows 1..111, zero pad
    w8 = _prep_weights(weights)
    in_maps = [
        {"x": xd[B_CORE * i : B_CORE * (i + 1)], "w": w8} for i in range(N_CORES)
    ]
    res = run_bass_kernel_spmd(
        nc, in_maps, core_ids=list(range(N_CORES)), trace=_trace
    )
    out = np.concatenate([res.results[i]["y"] for i in range(N_CORES)], axis=0)
    out = out.reshape(B_FULL, C_OUT, OH, OW).astype(np.float32)
    if _trace:
        return out, res
    return out


# revision 24
# speedup vs baseline: 1.8673x; 1.0425x over previous
"""Trainium2 Bass kernel: 3x3 VALID conv2d, stride 1.

Full input [32, 64, 112, 112] f32 + weights [128, 64, 3, 3] f32
-> output [32, 128, 110, 110] f32.

Data-parallel across 8 NeuronCores: 4 images per core.

Per-core formulation: conv as PE matmuls, out = lhsT.T @ rhs with
K (contraction, partitions) = 128 = 64 channels x 2 shifted copies,
M (out partitions) = 128 output channels,
N (moving free dim) = up to 4 input-width rows = 448 (<= 512, one PSUM
bank). The 2 rightmost columns of each 112-wide row are conv garbage;
the PSUM->SBUF copy compacts to the valid 110 columns.

Tap coverage per chunk, two schemes:
  T tile (all chunks): partitions 0-63 = image rows 0..111 (A),
          64-127 = rows 1..111 (B).  Matmuls m=0..2 at column offset
          kx apply tap pairs (0,kx)+(1,kx).
  5-MM scheme (chunks with y0 < 84): U tile: partitions 0-63 = rows
          2..88 (C), 64-127 = same shifted one column.  m=3 applies
          (2,0)+(2,1) in one full-K matmul; m=4 applies (2,2) on the
          hi half only -- 5 matmuls for 9 taps.
  6-MM scheme (y0 >= 84): ky=2 taps via T at row offset +1 with
          zero weights on the A half (planes 5-7) -- 6 matmuls, no U.

U is built on-device by two same-partition contiguous SBUF->SBUF DMAs
per band (lo: A shifted +224 elements; hi: B shifted +113), so HBM
input traffic stays at the single-copy ~12.9 MB/core.  Measured mover
rates force the hybrid: one HWDGE ring moves ~125 GB/s serialized, and
full-5MM needs ~140 GB/s of U-build; 75% coverage fits two rings.
U-lo copies ride the scalar ring, U-hi the sync ring, and the output
DMAs alternate between both rings to balance load; all U copies are
emitted in the group loop just before first use so an unfired T-band
semaphore never head-blocks other work on the queue.
(Also measured and rejected: streaming U from HBM saturates the
~358 GB/s HBM interface; compute-engine tensor_copy builds run at a
crawl, 14-25 G elem/s.)

Moving-N per tap is trimmed (n, n-1, n-2, ...) so no rhs read spills
past input row y0+3 (y0+4 for the 6-MM tail rows): only garbage output
columns lose taps.

Inputs are cast to fp16 on the host (fp32 PE is 4x slower; fp32 PSUM
accumulation keeps rel err ~4e-4).  Output is stored fp16 and cast
back to fp32 on the host, halving output HBM traffic.

A short burst of dummy matmuls on a memset tile runs during the DMA
startup window so the PE HAM clock gate flips to 2.4 GHz by the time
real work arrives.

Schedule: chunks are processed in groups of 8 across the 8 PSUM banks,
weight-plane-major (m outer), so consecutive matmuls hit different
banks (drain overlaps fill).  PSUM compaction casts alternate
vector/scalar per chunk.
"""

import numpy as np

B_FULL = 32
N_CORES = 8
B_CORE = B_FULL // N_CORES  # 4 images per core
C_IN = 64
C_OUT = 128
H = W = 112
OH = OW = 110
TLEN = 112 * W  # T plane: rows 0..111 (A) / 1..111 + zero pad (B)
Y5 = 64  # chunks with y0 < Y5 use the 5-matmul scheme
UROWS = Y5 + 3  # U plane rows 2..88
ULEN = UROWS * W

_NC = None


def _img_chunks():
    # per image: 27 chunks of 4 output rows + 1 of 2 rows = 110
    rows_list = [4] * 27 + [2]
    out = []
    y0 = 0
    for r in rows_list:
        out.append((y0, r))
        y0 += r
    assert y0 == OH
    return out


def _build():
    from contextlib import ExitStack

    import concourse.tile as tile
    from concourse import bacc, mybir

    nc = bacc.Bacc("TRN2", target_bir_lowering=False, debug=False)
    x = nc.dram_tensor(
        "x", [B_CORE, 128, TLEN], mybir.dt.float16, kind="ExternalInput"
    )
    w = nc.dram_tensor("w", [128, 8, 128], mybir.dt.float16, kind="ExternalInput")
    y = nc.dram_tensor(
        "y", [B_CORE, C_OUT, OH * OW], mybir.dt.float16, kind="ExternalOutput"
    )

    chunks = [(b, y0, r) for b in range(B_CORE) for (y0, r) in _img_chunks()]
    assert len(chunks) % 8 == 0
    n_groups = len(chunks) // 8

    with tile.TileContext(nc) as tc, ExitStack() as ctx:
        tpool = ctx.enter_context(tc.tile_pool(name="tp", bufs=B_CORE))
        upool = ctx.enter_context(tc.tile_pool(name="up", bufs=B_CORE))
        wpool = ctx.enter_context(tc.tile_pool(name="wp", bufs=1))
        spool = ctx.enter_context(tc.tile_pool(name="sp", bufs=1))
        opool = ctx.enter_context(tc.tile_pool(name="op", bufs=12))
        ppool = ctx.enter_context(tc.tile_pool(name="pp", bufs=8, space="PSUM"))

        wt = wpool.tile([128, 8 * 128], mybir.dt.float16)
        nc.sync.dma_start(wt[:], w.ap().rearrange("p a b -> p (a b)"))

        # PE warmup: HAM clock gate flips to 2.4 GHz after ~3.4us of
        # sustained activity; burn that in while the first x bands load.
        wu = spool.tile([128, 128], mybir.dt.float16)
        nc.gpsimd.memset(wu[:], 0)
        wu_p = ppool.tile([128, 448], mybir.dt.float32, name="wu_p", tag="pt")
        for _ in range(14):
            nc.tensor.matmul(
                wu_p[0:64, 0:128], wu[:, 0:64], wu[:],
                start=True, stop=True, skip_group_check=True,
            )

        xa = x.ap()
        ya = y.ap()

        # Banded loads so the first chunks start early.  Image 0's first
        # two T bands ride the sync queue (earliest to start); the bulk
        # of T streams on gpsimd/SWDGE.  U band k reads only T band k
        # (U edges = T edges - 2).
        t_tiles = [
            tpool.tile([128, TLEN], mybir.dt.float16, name=f"t{b}", tag="t")
            for b in range(B_CORE)
        ]
        u_tiles = [
            upool.tile([128, ULEN], mybir.dt.float16, name=f"u{b}", tag="u")
            for b in range(B_CORE)
        ]
        TBs = [[0, 6, 16, 26, 34, 61, 89, 112]] + [[0, 16, 34, 61, 89, 112]] * 3
        UBs = [[0, 4, 14, 24, 32, 59, UROWS]] + [[0, 14, 32, 59, UROWS]] * 3
        assert UROWS == 67

        def t_issue(b, k):
            lo, hi = TBs[b][k], TBs[b][k + 1]
            eng = nc.sync if (b == 0 and k < 3) else nc.gpsimd
            eng.dma_start(
                t_tiles[b][:, lo * W : hi * W], xa[b][:, lo * W : hi * W]
            )

        def u_build(b, k):
            # lo half rides the scalar ring (dedicated to U builds); the
            # hi half of image 0 rides sync's idle early window so image
            # 0's U is complete before its 5-MM groups need it
            lo, hi = UBs[b][k], UBs[b][k + 1]
            loeng = nc.scalar
            hieng = nc.sync if b == 0 else nc.scalar
            loeng.dma_start(
                u_tiles[b][0:64, lo * W : hi * W],
                t_tiles[b][0:64, (lo + 2) * W : (hi + 2) * W],
            )
            hieng.dma_start(
                u_tiles[b][64:128, lo * W : hi * W],
                t_tiles[b][64:128, lo * W + 113 : hi * W + 113],
            )

        for b in range(B_CORE):
            for k in range(len(TBs[b]) - 1):
                t_issue(b, k)
        # scalar's queue carries only U builds, so the in-order semaphore
        # waits (T band b,k) head-block nothing; emission order matches
        # T landing order for a natural pipeline
        for b in range(B_CORE):
            for k in range(len(UBs[b]) - 1):
                u_build(b, k)

        def chunk_taps(b, y0, rows):
            n = rows * W
            t, u = t_tiles[b], u_tiles[b]
            if y0 < Y5:  # 5-matmul scheme
                return [
                    (0, t, y0 * W, n),
                    (1, t, y0 * W + 1, n - 1),
                    (2, t, y0 * W + 2, n - 2),
                    (3, u, y0 * W, n - 1),
                    (4, u, y0 * W + 1, n - 2),
                ]
            return [  # 6-matmul scheme, ky=2 via T at +1 row
                (0, t, y0 * W, n),
                (1, t, y0 * W + 1, n - 1),
                (2, t, y0 * W + 2, n - 2),
                (5, t, (y0 + 1) * W, n),
                (6, t, (y0 + 1) * W + 1, n - 1),
                (7, t, (y0 + 1) * W + 2, n - 2),
            ]

        for g in range(n_groups):
            gchunks = chunks[g * 8 : (g + 1) * 8]
            pts = [
                ppool.tile([128, 448], mybir.dt.float32, name="pt", tag="pt")
                for _ in range(8)
            ]
            taps = [chunk_taps(*c) for c in gchunks]
            for m in range(8):
                for j in range(8):
                    for mi, (mm, src, off, nmv) in enumerate(taps[j]):
                        if mm != m:
                            continue
                        nc.tensor.matmul(
                            pts[j][:, 0:nmv],
                            wt[:, m * 128 : (m + 1) * 128],
                            src[:, off : off + nmv],
                            start=(mi == 0),
                            stop=(mi == len(taps[j]) - 1),
                            skip_group_check=True,
                        )
            # compact + store per 2 chunks: copies start draining PSUM as
            # soon as each pair of banks stops; out DMAs alternate between
            # the sync and scalar rings to balance ring load
            for h in range(4):
                pair = gchunks[2 * h : 2 * h + 2]
                total_rows = sum(r for _, _, r in pair)
                ot = opool.tile([128, 8 * OW], mybir.dt.float16, tag="ot")
                off = 0
                for jj, (b, y0, rows) in enumerate(pair):
                    j = 2 * h + jj
                    psrc = pts[j][:].rearrange("p (r c) -> p r c", c=W)[
                        :, 0:rows, 0:OW
                    ]
                    odst = ot[:, off : off + rows * OW].rearrange(
                        "p (r c) -> p r c", c=OW
                    )
                    # all casts on vector: it issues no DMAs, so PSUM
                    # drain is never head-blocked by an unfired T-band
                    # semaphore on a DMA-issuing queue
                    nc.vector.tensor_copy(odst, psrc)
                    off += rows * OW
                b0, y00, _ = pair[0]
                assert all(b == b0 for b, _, _ in pair)
                nc.sync.dma_start(
                    ya[b0][:, y00 * OW : y00 * OW + total_rows * OW],
                    ot[:, 0 : total_rows * OW],
                )

    nc.compile()
    return nc


def _get_nc():
    global _NC
    if _NC is None:
        _NC = _build()
    return _NC


def _prep_weights(weights: np.ndarray) -> np.ndarray:
    # planes 0-2: rows 0-63 = taps (0,m), rows 64-127 = taps (1,m)
    # plane 3: (2,0) | (2,1); plane 4: zero | (2,2)     [5-MM scheme]
    # planes 5-7: zero | (2,kx)                          [6-MM scheme]
    w = np.asarray(weights, dtype=np.float32)
    wt = w.transpose(1, 2, 3, 0)  # [ci, ky, kx, co]
    w8 = np.zeros((128, 8, 128), np.float32)
    w8[0:64, 0:3, :] = wt[:, 0, :, :]
    w8[64:128, 0:3, :] = wt[:, 1, :, :]
    w8[0:64, 3, :] = wt[:, 2, 0, :]
    w8[64:128, 3, :] = wt[:, 2, 1, :]
    w8[64:128, 4, :] = wt[:, 2, 2, :]
    w8[64:128, 5:8, :] = wt[:, 2, :, :]
    return w8.astype(np.float16)


def kernel(input_image: np.ndarray, weights: np.ndarray, _trace: bool = False):
    from concourse.bass_utils import run_bass_kernel_spmd

    nc = _get_nc()
    x16 = np.asarray(input_image).astype(np.float16)
    r = x16.reshape(B_FULL, C_IN, H * W)
    xd = np.zeros((B_FULL, 128, TLEN), np.float16)
    xd[:, 0:64] = r  # A: rows 0..111
    xd[:, 64:128, : TLEN - W] = r[:, :, W:]  # B: rows 1..111, zero pad
    w8 = _prep_weights(weights)
    in_maps = [
        {"x": xd[B_CORE * i : B_CORE * (i + 1)], "w": w8} for i in range(N_CORES)
    ]
    res = run_bass_kernel_spmd(
        nc, in_maps, core_ids=list(range(N_CORES)), trace=_trace
    )
    out = np.concatenate([res.results[i]["y"] for i in range(N_CORES)], axis=0)
    out = out.reshape(B_FULL, C_OUT, OH, OW).astype(np.float32)
    if _trace:
        return out, res
    return out
